# revision 1
# baseline (speedup 1.0000x reference)
"""Trainium2 Bass kernel for nn_Attention2d (sparse_attention).

Reference computation (per batch b=1):
    qkv = pair_act @ W_qkv.T + b_qkv              # [S,R,3D]
    q,k,v split, reshaped to heads [S,R,H,HD]
    logits[s,h,i,j] = q[s,i,h,:] . k[s,j,h,:]
    logits masked with attention_mask[s,j] -> -1e9
    attn = softmax_j(logits) * bias[h,i,j]
    o[s,i,:] = sum_j attn . v  -> out = o @ W_out.T + b_out

Sharding: data-parallel over S (32 rows -> 4 per core, 8 cores). Each core
computes its 4 rows fully (all heads); no collectives.

Per-core layout ("transposed attention"): keys j live on SBUF partitions so
  - logitsT[j,i] accumulates in PSUM straight from the PE,
  - exp() is a single ACT pass reading PSUM,
  - softmax denominators come from a masked-ones matmul (sum over partitions),
  - the o-matmul consumes P^T directly (contraction dim on partitions).
The mask is applied by zeroing masked rows of v and of the ones-vector
(exactly equivalent to the -1e9 bias: exp(-1e9) == 0 in fp32).
The [384,384] per-head bias is DMA-transposed (bf16 xbar transpose).

The matmul operands (x, bias, W_qkv, W_out) are cast to bf16 ON THE HOST
and shipped as one flat bf16 tensor, so the DMA transposes read straight
from the input: no on-device casts, no DRAM roundtrip (~5.3MB HBM traffic
per core instead of ~14.8MB).  A tiny fp32 tensor carries b_qkv/b_out and
the precomputed mask01.  Built on bacc.Bacc (+ nc.compile()): its
nop-fusion legalizes multi-wait instructions that walrus codegen otherwise
rejects ("Too many sync wait commands" - DmaTransposeAnt has one wait slot).
"""

import ml_dtypes
import numpy as np

import concourse.bass as bass
import concourse.tile as tile
import concourse.mybir as mybir
from concourse import bacc
from concourse.bass_utils import run_bass_kernel_spmd

# Problem shapes (hardcoded per contract; kernel.py must be self-contained).
B, S, R, D = 1, 32, 384, 256
H, HD = 8, 32
NCORES = 8
SS = S // NCORES          # 4 sequence rows per core
M = SS * R                # 1536 flattened rows per core
MT = M // 128             # 12 m-tiles
JT = R // 128             # 3 j-tiles per row
KT = D // 128             # 2 k-tiles of the model dim
F32 = mybir.dt.float32
BF16 = mybir.dt.bfloat16
AF = mybir.ActivationFunctionType
ALU = mybir.AluOpType

# Flat packing offsets of the bf16 input tensor (bf16 elements).
BOFF_X = 0                             # [M, D]
BOFF_BIAS = BOFF_X + M * D             # [H*R, R]
BOFF_WQKV = BOFF_BIAS + H * R * R      # [3D, D]
BOFF_WOUT = BOFF_WQKV + 3 * D * D      # [D, D]
NBF = BOFF_WOUT + D * D
# Flat packing offsets of the small fp32 input tensor (fp32 elements).
FOFF_BQKV = 0                          # [3D]
FOFF_BOUT = FOFF_BQKV + 3 * D          # [D]
FOFF_M01 = FOFF_BOUT + D               # [SS, R] mask01 (1.0 = keep)
NF32 = FOFF_M01 + SS * R


def build_program(zero_bias: bool = False) -> bass.Bass:
    nc = bacc.Bacc("TRN2", target_bir_lowering=False, debug=False,
                   num_devices=NCORES)
    allin_bf = nc.dram_tensor("allin_bf", [NBF], BF16, kind="ExternalInput")
    allin_f32 = nc.dram_tensor("allin_f32", [NF32], F32,
                               kind="ExternalInput")
    out_dram = nc.dram_tensor("out", [M, D], F32, kind="ExternalOutput")
    with tile.TileContext(nc) as tc:
        _emit(nc, tc, allin_bf, allin_f32, out_dram, zero_bias)
    nc.compile()
    return nc


def _emit(nc, tc, allin_bf, allin_f32, out_dram, zero_bias):
    from contextlib import ExitStack
    ctx = ExitStack()
    with ctx:
        singles = ctx.enter_context(tc.tile_pool(name="singles", bufs=1))

        # ---- Phase 0: small fp32 loads + transposed bf16 reads ----
        xbf = allin_bf[BOFF_X:BOFF_BIAS].rearrange("(r c) -> r c", c=D)
        biasbf = allin_bf[BOFF_BIAS:BOFF_WQKV].rearrange("(r c) -> r c", c=R)
        wqbf = allin_bf[BOFF_WQKV:BOFF_WOUT].rearrange("(r c) -> r c", c=D)
        wobf = allin_bf[BOFF_WOUT:NBF].rearrange("(r c) -> r c", c=D)

        # Per-kt tiles so each consumer matmul unblocks on its own
        # transpose rather than the whole pair.
        xT0 = singles.tile([128, M], BF16, tag="xT0")
        xT1 = singles.tile([128, M], BF16, tag="xT1")
        xT = [xT0, xT1]
        wqT0 = singles.tile([128, 3 * D], BF16, tag="wqT0")
        wqT1 = singles.tile([128, 3 * D], BF16, tag="wqT1")
        wqT = [wqT0, wqT1]
        nc.sync.dma_start(out=xT[0][:], in_=xbf[:, 0:128], transpose=True)
        nc.sync.dma_start(out=wqT[0][:], in_=wqbf[:, 0:128], transpose=True)
        nc.sync.dma_start(out=xT[1][:], in_=xbf[:, 128:256], transpose=True)
        nc.sync.dma_start(out=wqT[1][:], in_=wqbf[:, 128:256], transpose=True)
        bq_sb = singles.tile([128, 4], F32)
        mb01 = singles.tile([128, SS, JT], F32)
        nc.sync.dma_start(
            out=bq_sb[:],
            in_=allin_f32[FOFF_BQKV:FOFF_BQKV + 512]
            .rearrange("(t p) -> p t", p=128))
        nc.sync.dma_start(
            out=mb01[:],
            in_=allin_f32[FOFF_M01:FOFF_M01 + SS * R]
            .rearrange("(s t p) -> p s t", p=128, t=JT))
        if not zero_bias:
            bv_f32 = singles.tile([1, D], F32)
            bo_f32 = singles.tile([1, D], F32)
            nc.sync.dma_start(
                out=bv_f32[:],
                in_=allin_f32[FOFF_BQKV + 2 * D:FOFF_BQKV + 3 * D]
                .rearrange("(a b) -> a b", a=1))
            nc.sync.dma_start(
                out=bo_f32[:],
                in_=allin_f32[FOFF_BOUT:FOFF_BOUT + D]
                .rearrange("(a b) -> a b", a=1))

        biasT = singles.tile([128, JT, H * R], BF16)
        for jt in range(JT):
            nc.sync.dma_start(out=biasT[:, jt, :],
                              in_=biasbf[:, jt * 128:(jt + 1) * 128],
                              transpose=True)
        woT = singles.tile([128, KT, D], BF16)
        for kt in range(KT):
            nc.sync.dma_start(out=woT[:, kt, :],
                              in_=wobf[:, kt * 128:(kt + 1) * 128],
                              transpose=True)

        # small bf16 helper tiles
        if not zero_bias:
            bv_bf = singles.tile([1, D], BF16)
            nc.vector.tensor_copy(bv_bf[:], bv_f32[:])
            bo_bf = singles.tile([1, D], BF16)
            nc.vector.tensor_copy(bo_bf[:], bo_f32[:])
            ones_k1 = singles.tile([1, 128], BF16)
            nc.vector.memset(ones_k1[:], 1.0)
        ones32 = singles.tile([128, 32], BF16)
        nc.vector.memset(ones32[:], 1.0)
        # mask01 replicated over 32 columns, bf16 (ones-matmul stationary op)
        m01rep = singles.tile([128, SS, JT, 32], BF16)
        for s in range(SS):
            for jt in range(JT):
                nc.vector.tensor_scalar_mul(m01rep[:, s, jt, :], ones32[:],
                                            mb01[:, s, jt:jt + 1])

        # ---- Phase 1: qkv projection ----
        # qkT[n, m] for n in q(0:256)|k(256:512): 4 n-tiles
        qkT = singles.tile([128, 4, M], BF16)
        # v[m, d] natural layout
        vsb = singles.tile([128, MT, D], BF16)
        with tc.tile_pool(name="ps_qk", bufs=5, space="PSUM") as ps_qk, \
             tc.tile_pool(name="ps_v", bufs=3, space="PSUM") as ps_v:
            def emit_qk(nt):
                for mc in range(3):  # m in chunks of 512
                    pqk = ps_qk.tile([128, 512], F32, tag="pqk")
                    for kt in range(KT):
                        nc.tensor.matmul(
                            pqk[:],
                            wqT[kt][:, nt * 128:(nt + 1) * 128],
                            xT[kt][:, mc * 512:(mc + 1) * 512],
                            start=(kt == 0), stop=(kt == KT - 1))
                    dst = qkT[:, nt, mc * 512:(mc + 1) * 512]
                    if (nt + mc) % 2 == 0:
                        nc.vector.tensor_scalar_add(dst, pqk[:],
                                                    bq_sb[:, nt:nt + 1])
                    else:
                        nc.scalar.activation(dst, pqk[:], AF.Identity,
                                             bias=bq_sb[:, nt:nt + 1])
            # head-group 0 consumes q/k tiles 0 and 2 + s=0's v tiles:
            # emit those first so attention starts mid-projection.
            emit_qk(0)
            emit_qk(2)
            def emit_v(mt):
                pv = ps_v.tile([128, D], F32, tag="pv")
                if not zero_bias:
                    # b_v broadcast preload, accumulated under the matmuls
                    nc.tensor.matmul(pv[:], ones_k1[:], bv_bf[:],
                                     start=True, stop=False)
                for kt in range(KT):
                    nc.tensor.matmul(
                        pv[:],
                        xT[kt][:, mt * 128:(mt + 1) * 128],
                        wqT[kt][:, 2 * D:3 * D],
                        start=(zero_bias and kt == 0), stop=(kt == KT - 1))
                if mt % 2 == 0:
                    nc.vector.tensor_copy(vsb[:, mt, :], pv[:])
                else:
                    nc.scalar.copy(vsb[:, mt, :], pv[:])
            for mt in (0, 1, 2):
                emit_v(mt)
            emit_qk(1)
            emit_qk(3)
            for mt in range(3, MT):
                emit_v(mt)

        # ---- Phase 2: attention per (s, head-group) ----
        # oT[d, (s,i)]: normalized attention output, transposed for out-proj
        oT = singles.tile([128, KT, M], BF16)
        work = ctx.enter_context(tc.tile_pool(name="work", bufs=2))
        pt_pool = ctx.enter_context(tc.tile_pool(name="pt", bufs=10))
        pbt_pool = ctx.enter_context(tc.tile_pool(name="pbt", bufs=8))
        rec_pool = ctx.enter_context(tc.tile_pool(name="rec", bufs=2))
        outf_pool = ctx.enter_context(tc.tile_pool(name="outf", bufs=3))
        with tc.tile_pool(name="ps_lg", bufs=2, space="PSUM") as ps_lg, \
             tc.tile_pool(name="ps_den", bufs=1, space="PSUM") as ps_den, \
             tc.tile_pool(name="ps_o", bufs=1, space="PSUM") as ps_o:
            for s in range(SS):
                # masked v for this row: zero out masked j rows
                vmask = work.tile([128, JT, D], BF16)
                for jt in range(JT):
                    nc.vector.tensor_scalar_mul(
                        vmask[:, jt, :], vsb[:, s * JT + jt, :],
                        mb01[:, s, jt:jt + 1])
                for g in range(2):  # head groups of 4
                    pts = []
                    pbts = []
                    for hp in range(4):
                        h = 4 * g + hp
                        # logitsT[j, i] = k^T . q ; 512-strided PSUM banks
                        lg = ps_lg.tile([128, JT, 512], F32)
                        for jt in range(JT):
                            nc.tensor.matmul(
                                lg[:, jt, 0:R],
                                qkT[32 * hp:32 * hp + 32, 2 + g,
                                    s * R + jt * 128: s * R + (jt + 1) * 128],
                                qkT[32 * hp:32 * hp + 32, g,
                                    s * R:(s + 1) * R],
                                start=True, stop=True,
                                tile_position=(32 * hp, 0))
                        # P^T = exp(logitsT), one ACT pass over strided banks
                        pt = pt_pool.tile([128, JT, R], BF16)
                        nc.scalar.activation(pt[:], lg[:, :, 0:R], AF.Exp)
                        pts.append(pt)
                        # biased attention weights: P^T * bias^T (bf16 2x DVE)
                        pbt = pbt_pool.tile([128, JT, R], BF16)
                        nc.vector.tensor_mul(
                            pbt[:], pt[:],
                            biasT[:, :, h * R:(h + 1) * R])
                        pbts.append(pbt)
                    # softmax denominators: masked-ones matmul, 4 heads
                    # col-packed into one bank, each head's sum replicated
                    # over its 32-partition block.
                    den = ps_den.tile([128, R], F32)
                    for hp in range(4):
                        for jt in range(JT):
                            nc.tensor.matmul(
                                den[32 * hp:32 * hp + 32, :],
                                m01rep[:, s, jt, :],
                                pts[hp][:, jt, :],
                                start=(jt == 0), stop=(jt == JT - 1),
                                tile_position=(0, 32 * hp))
                    # o^T[d, i] for the 4 heads, col-packed into one bank
                    po = ps_o.tile([128, R], F32)
                    for hp in range(4):
                        h = 4 * g + hp
                        for jt in range(JT):
                            nc.tensor.matmul(
                                po[32 * hp:32 * hp + 32, :],
                                vmask[:, jt, h * HD:(h + 1) * HD],
                                pbts[hp][:, jt, :],
                                start=(jt == 0), stop=(jt == JT - 1),
                                tile_position=(0, 32 * hp))
                    rec = rec_pool.tile([128, R], F32)
                    nc.vector.reciprocal(rec[:], den[:])
                    nc.vector.tensor_mul(oT[:, g, s * R:(s + 1) * R],
                                         po[:], rec[:])
                # output projection for this row, overlapping the next
                # row's attention; psum borrowed from the den slot.
                for mt in range(3 * s, 3 * s + 3):
                    # alternate between the den and po slots so the next
                    # row's denominator/o matmuls stall behind at most one
                    # projection tile each
                    if mt % 2 == 0:
                        pf = ps_den.tile([128, D], F32, tag="den")
                    else:
                        pf = ps_o.tile([128, D], F32, tag="po")
                    if not zero_bias:
                        nc.tensor.matmul(pf[:], ones_k1[:], bo_bf[:],
                                         start=True, stop=False)
                    for kt in range(KT):
                        nc.tensor.matmul(
                            pf[:],
                            oT[:, kt, mt * 128:(mt + 1) * 128],
                            woT[:, kt, :],
                            start=(zero_bias and kt == 0),
                            stop=(kt == KT - 1))
                    fo = outf_pool.tile([128, D], F32)
                    if mt % 2 == 0:
                        nc.vector.tensor_copy(fo[:], pf[:])
                    else:
                        nc.scalar.copy(fo[:], pf[:])
                    nc.sync.dma_start(
                        out=out_dram[mt * 128:(mt + 1) * 128, :], in_=fo[:])



def make_in_maps(pair_act, attention_mask, bias, W_qkv, b_qkv, W_out, b_out):
    """Shard the full inputs across the 8 cores (data-parallel over S).

    Matmul operands are cast to bf16 host-side (round-to-nearest-even, same
    as the on-device cast) so the kernel never touches fp32 copies of them.
    """
    bf = ml_dtypes.bfloat16
    pair_bf = np.asarray(pair_act, dtype=np.float32).astype(bf)
    shared_bf = np.concatenate([
        np.asarray(bias, np.float32).ravel().astype(bf),
        np.asarray(W_qkv, np.float32).ravel().astype(bf),
        np.asarray(W_out, np.float32).ravel().astype(bf)])
    mask01 = 1.0 - np.asarray(attention_mask).astype(np.float32)  # 1 = keep
    f32part = np.concatenate([
        np.asarray(b_qkv, np.float32).ravel(),
        np.asarray(b_out, np.float32).ravel()])
    in_maps = []
    for c in range(NCORES):
        sl = slice(c * SS, (c + 1) * SS)
        allin_bf = np.concatenate([pair_bf[0, sl].ravel(), shared_bf])
        allin_f32 = np.concatenate([f32part, mask01[0, sl].ravel()])
        assert allin_bf.size == NBF and allin_f32.size == NF32
        in_maps.append({
            "allin_bf": np.ascontiguousarray(allin_bf),
            "allin_f32": np.ascontiguousarray(allin_f32.astype(np.float32)),
        })
    return in_maps


_PROGRAM_CACHE = {}


def kernel(pair_act, attention_mask, bias, W_qkv, b_qkv, W_out, b_out,
           _want_results=False, **extra):
    in_maps = make_in_maps(pair_act, attention_mask, bias, W_qkv, b_qkv,
                           W_out, b_out)
    zero_bias = bool(np.all(np.asarray(b_qkv) == 0)
                     and np.all(np.asarray(b_out) == 0))
    key = ("nc", zero_bias)
    if key not in _PROGRAM_CACHE:
        _PROGRAM_CACHE[key] = build_program(zero_bias)
    nc = _PROGRAM_CACHE[key]
    res = run_bass_kernel_spmd(nc, in_maps, core_ids=list(range(NCORES)))
    out = np.concatenate(
        [r["out"].reshape(SS, R, D) for r in res.results], axis=0)
    out = out.reshape(B, S, R, D).astype(np.float32)
    if _want_results:
        return out, res
    return out

